# revision 1
# baseline (speedup 1.0000x reference)
"""BasicTransformerBlock on 8 TRN2 NeuronCores.

Strategy: pure data parallelism over the batch (B=8, one batch element per
core). Inside each core the block runs with feature-major activations:
  - all dense projections as fp32r matmuls (lhsT = weight in natural [K,M]
    DRAM layout, rhs = feature-major activations)
  - attention score/AV and the output projections / FF2 in bf16
  - softmax denominator via a ones-column appended to V (lands as an extra
    PSUM partition row), normalized with a fast-reciprocal after a
    DRAM-roundtrip partition broadcast
  - layernorm stats via ones-vector matmuls (partition reduction on the PE)
The host pre-transposes x/context, reshapes biases per-partition, and
transposes the output back.
"""

import math

import numpy as np
import ml_dtypes

import concourse.bass as bass
import concourse.mybir as mybir
import concourse.tile as tile
from concourse import bacc
from concourse.bass_utils import run_bass_kernel_spmd

F32 = mybir.dt.float32
F32R = mybir.dt.float32r
BF16 = mybir.dt.bfloat16
AF = mybir.ActivationFunctionType
OP = mybir.AluOpType

P = 128
B = 8
NT = 1024          # query tokens
D = 640            # model dim; 5 chunks of 128
KC = 5
NH = 8             # heads
DH = 80            # head dim
CM = 77            # context tokens
CD = 768           # context dim; 6 chunks
CKC = 6
FH = 2560          # GEGLU half hidden; 20 chunks of 128
FJ = 20
NC = 2             # token chunks of 512
NW = 512
ISCALE = 1.0 / math.sqrt(DH)
LN_EPS = 1e-5


def _emit(nc, tc, apply_gb):
    d = nc._kd  # dram handles dict
    with (
        tc.tile_pool(name="sb", bufs=1) as sb,
        tc.tile_pool(name="ps", bufs=1, space="PSUM") as ps,
    ):
        _emit_body(nc, tc, d, sb, ps, apply_gb)


def _emit_body(nc, tc, d, sb, ps, apply_gb):
    def bank(name):
        return ps.tile([P, NW], F32, tag="bank", bufs=8, name=name)

    # ---------------- critical-path loads first: xT + wq + wk per chunk ----
    def resid_tile(name):
        return sb.tile([P, KC, NT], F32R, tag="resid", bufs=2, name=name)

    xT = resid_tile("xT")
    w_sa_q = sb.tile([P, KC, D], F32R, tag="w640", bufs=4, name="w_sa_q")
    w_sa_k = sb.tile([P, KC, D], F32R, tag="w640", bufs=4, name="w_sa_k")
    for c in range(KC):
        nc.sync.dma_start(xT[:, c, :],
                          d["xT"].rearrange("(c p) n -> p c n", p=P)[:, c, :])
        nc.gpsimd.dma_start(
            w_sa_q[:, c, :],
            d["sa_wq"].rearrange("(c p) m -> p c m", p=P)[:, c, :])
        nc.gpsimd.dma_start(
            w_sa_k[:, c, :],
            d["sa_wk"].rearrange("(c p) m -> p c m", p=P)[:, c, :])

    # ---------------- constant / bias tiles (gpsimd SWDGE queue) ----------
    ones_r = sb.tile([P, 1], F32R, tag="ones", name="ones_r")
    nc.gpsimd.dma_start(ones_r, d["ones"][:, :])
    epst = sb.tile([P, 1], F32, tag="epst", name="epst")
    nc.gpsimd.dma_start(epst, d["epst"][:, :])
    b_sa_bo = sb.tile([P, KC], F32, tag="b1", name="b_sa_bo")
    nc.gpsimd.dma_start(b_sa_bo, d["sa_bo_p"][:, :])
    b_ca_bo = sb.tile([P, KC], F32, tag="b2", name="b_ca_bo")
    nc.gpsimd.dma_start(b_ca_bo, d["ca_bo_p"][:, :])
    b_ff2 = sb.tile([P, KC], F32, tag="b3", name="b_ff2")
    nc.gpsimd.dma_start(b_ff2, d["ff_b2_p"][:, :])
    b_f1a = sb.tile([P, FJ], F32, tag="b4", name="b_f1a")
    nc.gpsimd.dma_start(b_f1a, d["ff_b1a_p"][:, :])
    b_f1g = sb.tile([P, FJ], F32, tag="b5", name="b_f1g")
    nc.gpsimd.dma_start(b_f1g, d["ff_b1g_p"][:, :])
    lngb = {}
    if apply_gb:
        for ln in (1, 2, 3):
            for gb in ("g", "b"):
                t = sb.tile([P, KC], F32, tag=f"ln{ln}{gb}", name=f"ln{ln}{gb}")
                nc.gpsimd.dma_start(t, d[f"ln{ln}_{gb}_p"][:, :])
                lngb[(ln, gb)] = t

    ctxT = sb.tile([P, CKC, CM], BF16, tag="ctxT", bufs=1, name="ctxT")
    nc.gpsimd.dma_start(ctxT, d["ctxT_bf"].rearrange("(c p) m -> p c m", p=P))

    def w640(key, name, split=False):
        t = sb.tile([P, KC, D], F32R, tag="w640", bufs=4, name=name)
        nc.gpsimd.dma_start(t, d[key].rearrange("(c p) m -> p c m", p=P))
        return t

    w_sa_v = sb.tile([P, KC, D], F32R, tag="w640", bufs=4, name="w_sa_v")
    for c in range(KC):
        nc.sync.dma_start(
            w_sa_v[:, c, :],
            d["sa_wv"].rearrange("(c p) m -> p c m", p=P)[:, c, :])

    w_sa_o = sb.tile([DH, NH, D], BF16, tag="wo", bufs=1, name="w_sa_o")
    nc.gpsimd.dma_start(w_sa_o, d["sa_wo_h"][:, :, :])

    def qk_proj(h, w_q, w_k, src, pref):
        hs = slice(h * DH, (h + 1) * DH)
        qt = sb.tile([DH, NT], BF16, tag="qk", bufs=2, name=f"qt{pref}_{h}")
        kt = sb.tile([DH, NT], BF16, tag="qk", bufs=2, name=f"kt{pref}_{h}")
        for ncq in range(NC):
            ncs = slice(ncq * NW, (ncq + 1) * NW)
            pq = ps.tile([DH, NW], F32, tag="bank", bufs=8,
                         name=f"pq{pref}_{h}_{ncq}")
            pk = ps.tile([DH, NW], F32, tag="bank", bufs=8,
                         name=f"pk{pref}_{h}_{ncq}")
            for c in range(KC):
                nc.tensor.matmul(pq, w_q[:, c, hs], src[:, c, ncs],
                                 start=(c == 0), stop=(c == KC - 1))
            for c in range(KC):
                nc.tensor.matmul(pk, w_k[:, c, hs], src[:, c, ncs],
                                 start=(c == 0), stop=(c == KC - 1))
            nc.vector.tensor_copy(qt[:, ncs], pq)
            nc.vector.tensor_copy(kt[:, ncs], pk)
        return qt, kt

    # head-0 Q/K first: fills the PE while V weights land
    qk0 = qk_proj(0, w_sa_q, w_sa_k, xT, "s")

    # ---------------- SA: V projection into V_aug ----------------
    v_aug = sb.tile([P, NH, NH, 97], BF16, tag="vaug", bufs=1, name="v_aug")
    nc.vector.memset(v_aug[:, :, :, 80:96], 0.0)
    nc.vector.memset(v_aug[:, :, :, 96:97], 1.0)
    for tch in range(NH):
        for g in range(2):  # two groups of 4 head-columns (320 wide)
            pv = ps.tile([P, 320], F32, tag="bank", bufs=8, name=f"pv_{tch}_{g}")
            for c in range(KC):
                nc.tensor.matmul(
                    pv, xT[:, c, tch * P:(tch + 1) * P],
                    w_sa_v[:, c, g * 320:(g + 1) * 320],
                    start=(c == 0), stop=(c == KC - 1))
            nc.vector.tensor_copy(
                v_aug[:, tch, 4 * g:4 * g + 4, 0:80],
                pv.rearrange("p (s e) -> p s e", e=80))

    # scratch DRAM for partition broadcasts
    zdram = nc.dram_tensor("zdram", [2 * NH * NC, NW], F32)
    sdram = nc.dram_tensor("sdram", [3, 2, NC, NW], F32)

    def attn_inner(h, qt, kt, vaug_sl, o_tile, mchunks, mpart, zslot,
                   pe_bcast=False):
        """scores -> exp -> AV -> normalize for one (head, nc-chunk) pair.

        vaug_sl(mc) -> lhsT [mpart, 97]; o_tile[:, h, ncs] gets bf16 O_norm.
        """
        for ncq in range(NC):
            ncs = slice(ncq * NW, (ncq + 1) * NW)
            po = ps.tile([97, NW], F32, tag="bank", bufs=8, name=f"po_{h}_{ncq}")
            ets = []
            for mc in range(mchunks):
                pscore = ps.tile([mpart, NW], F32, tag="bank", bufs=8,
                                 name=f"psc_{h}_{ncq}_{mc}")
                nc.tensor.matmul(pscore, kt(mc), qt[:, ncs],
                                 start=True, stop=True)
                et = sb.tile([mpart, NW], BF16, tag="epool", bufs=4,
                             name=f"e_{h}_{ncq}_{mc}")
                nc.scalar.activation(et, pscore, AF.Exp, scale=ISCALE)
                ets.append(et)
                if mc >= 1:  # AV one score behind its exp
                    nc.tensor.matmul(po, vaug_sl(mc - 1), ets[mc - 1],
                                     start=(mc == 1), stop=False,
                                     skip_group_check=True)
            last = mchunks - 1
            nc.tensor.matmul(po, vaug_sl(last), ets[last],
                             start=(mchunks == 1), stop=True,
                             skip_group_check=True)
            # softmax denominator -> broadcast -> reciprocal -> normalize
            slot = zslot + ncq
            zb = sb.tile([DH, NW], F32, tag="zb", bufs=2, name=f"zb_{h}_{ncq}")
            if pe_bcast:
                zrow_r = sb.tile([1, NW], F32R, tag="zrow", bufs=4,
                                 name=f"zr_{h}_{ncq}")
                nc.vector.tensor_copy(zrow_r, po[96:97, :])
                pzb = ps.tile([DH, NW], F32, tag="bank", bufs=8,
                              name=f"pzb_{h}_{ncq}")
                nc.tensor.matmul(pzb, ones_row[:, 0:DH], zrow_r,
                                 start=True, stop=True)
                nc.vector.reciprocal_approx_fast(zb, pzb)
            else:
                zrow = sb.tile([1, NW], F32, tag="zrow", bufs=4,
                               name=f"zr_{h}_{ncq}")
                nc.vector.tensor_copy(zrow, po[96:97, :])
                nc.sync.dma_start(zdram[slot:slot + 1, :], zrow)
                nc.sync.dma_start(
                    zb, zdram[slot:slot + 1, :].to_broadcast((DH, NW)))
                nc.vector.reciprocal_approx_fast(zb, zb)
            nc.vector.tensor_tensor(o_tile[:, h, ncs], po[0:80, :], zb,
                                    OP.mult)

    # ---------------- SA: per-head QK + attention ----------------
    o_sa = sb.tile([DH, NH, NT], BF16, tag="opool", bufs=1, name="o_sa")
    for h in range(NH):
        qt, kt = qk0 if h == 0 else qk_proj(h, w_sa_q, w_sa_k, xT, "s")
        attn_inner(
            h, qt,
            kt=lambda mc, _kt=kt: _kt[:, mc * P:(mc + 1) * P],
            vaug_sl=lambda mc, _h=h: v_aug[:, mc, _h, :],
            o_tile=o_sa, mchunks=NH, mpart=P, zslot=h * NC)

    def out_proj(ncq, wo_t, o_tile, bo_t, res_in, res_out, pref):
        ncs = slice(ncq * NW, (ncq + 1) * NW)
        for do in range(KC):
            dos = slice(do * P, (do + 1) * P)
            pr = bank(f"prj_{pref}_{do}_{ncq}")
            for h in range(NH):
                nc.tensor.matmul(pr, wo_t[:, h, dos], o_tile[:, h, ncs],
                                 start=(h == 0), stop=(h == NH - 1))
            nc.vector.scalar_tensor_tensor(
                out=res_out[:, do, ncs], in0=pr, scalar=bo_t[:, do:do + 1],
                in1=res_in[:, do, ncs].bitcast(F32), op0=OP.add, op1=OP.add)

    # ---------------- layernorm (feature-major) ----------------
    def layernorm_nc(rT, ln_idx, ncq):
        outT = rT
        sidx = ln_idx - 1
        if True:
            ncs = slice(ncq * NW, (ncq + 1) * NW)
            psum_s = ps.tile([1, NW], F32, tag="bank", bufs=8,
                             name=f"ls_{ln_idx}_{ncq}")
            psum_q = ps.tile([1, NW], F32, tag="bank", bufs=8,
                             name=f"lq_{ln_idx}_{ncq}")
            for c in range(KC):
                sq = sb.tile([P, NW], F32R, tag="sq", bufs=1,
                             name=f"sq_{ln_idx}_{ncq}_{c}")
                nc.scalar.activation(sq, rT[:, c, ncs].bitcast(F32),
                                     AF.Square)
                nc.tensor.matmul(psum_s, ones_r, rT[:, c, ncs],
                                 start=(c == 0), stop=(c == KC - 1))
                nc.tensor.matmul(psum_q, ones_r, sq,
                                 start=(c == 0), stop=(c == KC - 1))
            srow = sb.tile([1, NW], F32, tag="zrow", bufs=4, name=f"sr_{sidx}_{ncq}")
            nc.any.tensor_copy(srow, psum_s)
            nc.sync.dma_start(sdram[sidx, 0, ncq, :].unsqueeze(0), srow)
            qrow = sb.tile([1, NW], F32, tag="zrow", bufs=4, name=f"qr_{sidx}_{ncq}")
            nc.any.tensor_copy(qrow, psum_q)
            nc.sync.dma_start(sdram[sidx, 1, ncq, :].unsqueeze(0), qrow)
            mu_b = sb.tile([P, NW], F32, tag="mu_b", bufs=2,
                           name=f"mu_{ln_idx}_{ncq}")
            nc.sync.dma_start(
                mu_b, sdram[sidx, 0, ncq, :].unsqueeze(0).to_broadcast((P, NW)))
            wk = sb.tile([P, NW], F32, tag="wk_b", bufs=2, name=f"wk_{ln_idx}_{ncq}")
            nc.sync.dma_start(
                wk, sdram[sidx, 1, ncq, :].unsqueeze(0).to_broadcast((P, NW)))
            # mu = sum/D ; var = sq/D - mu^2 ; rstd = 1/sqrt(var+eps)
            nc.vector.tensor_scalar_mul(mu_b, mu_b, 1.0 / D)
            t2 = sb.tile([P, NW], F32, tag="t2_b", bufs=1, name=f"t2_{ln_idx}_{ncq}")
            nc.vector.tensor_tensor(t2, mu_b, mu_b, OP.mult)
            nc.vector.scalar_tensor_tensor(
                out=wk, in0=wk, scalar=1.0 / D, in1=t2,
                op0=OP.mult, op1=OP.subtract)
            nc.scalar.activation(wk, wk, AF.Sqrt, bias=epst)
            nc.vector.reciprocal_approx_fast(wk, wk)   # wk = rstd
            nc.vector.tensor_tensor(mu_b, mu_b, wk, OP.mult)  # mu_b = mu*rstd
            for c in range(KC):
                t1 = sb.tile([P, NW], F32, tag="t1_b", bufs=2,
                             name=f"t1_{ln_idx}_{ncq}_{c}")
                nc.vector.tensor_tensor(t1, rT[:, c, ncs].bitcast(F32), wk,
                                        OP.mult)
                if apply_gb:
                    nc.vector.tensor_tensor(t1, t1, mu_b, OP.subtract)
                    nc.vector.tensor_scalar(
                        out=outT[:, c, ncs], in0=t1,
                        scalar1=lngb[(ln_idx, "g")][:, c:c + 1],
                        scalar2=lngb[(ln_idx, "b")][:, c:c + 1],
                        op0=OP.mult, op1=OP.add)
                else:
                    tt = nc.vector.tensor_tensor(outT[:, c, ncs], t1, mu_b,
                                                 OP.subtract)

    # ---------------- SA out-proj + LN1, interleaved per token chunk ---------
    r1T = resid_tile("r1T")
    for ncq in range(NC):
        out_proj(ncq, w_sa_o, o_sa, b_sa_bo, xT, r1T, "r1")
        layernorm_nc(r1T, 1, ncq)
    x1T = r1T

    # ---------------- CA weights ----------------
    w_ca_q = w640("ca_wq", "w_ca_q")
    w_ca_k = sb.tile([P, CKC, D], BF16, tag="w768", bufs=1, name="w_ca_k")
    nc.gpsimd.dma_start(w_ca_k, d["ca_wk"].rearrange("(c p) m -> p c m", p=P))
    w_ca_v = sb.tile([P, CKC, D], BF16, tag="w768", bufs=1, name="w_ca_v")
    nc.gpsimd.dma_start(w_ca_v, d["ca_wv"].rearrange("(c p) m -> p c m", p=P))
    w_ca_o = sb.tile([DH, NH, D], BF16, tag="wo", bufs=1, name="w_ca_o")
    nc.gpsimd.dma_start(w_ca_o, d["ca_wo_h"][:, :, :])

    # ---------------- CA: K/V projections ----------------
    kt_ca = sb.tile([DH, NH, CM], BF16, tag="ktca", bufs=1, name="kt_ca")
    for h in range(NH):
        hs = slice(h * DH, (h + 1) * DH)
        pk = ps.tile([DH, CM], F32, tag="bank", bufs=8, name=f"pkca_{h}")
        for c in range(CKC):
            nc.tensor.matmul(pk, w_ca_k[:, c, hs], ctxT[:, c, :],
                             start=(c == 0), stop=(c == CKC - 1))
        nc.any.tensor_copy(kt_ca[:, h, :], pk)

    vca_aug = sb.tile([CM, NH, 97], BF16, tag="vca", bufs=1, name="vca_aug")
    nc.vector.memset(vca_aug[:, :, 80:96], 0.0)
    nc.vector.memset(vca_aug[:, :, 96:97], 1.0)
    for g in range(2):
        pv = ps.tile([CM, 320], F32, tag="bank", bufs=8, name=f"pvca_{g}")
        for c in range(CKC):
            nc.tensor.matmul(pv, ctxT[:, c, :],
                             w_ca_v[:, c, g * 320:(g + 1) * 320],
                             start=(c == 0), stop=(c == CKC - 1))
        nc.any.tensor_copy(vca_aug[:, 4 * g:4 * g + 4, 0:80],
                           pv.rearrange("p (s e) -> p s e", e=80))

    # ---------------- CA: per-head Q + attention ----------------
    o_ca = sb.tile([DH, NH, NT], BF16, tag="opool", bufs=1, name="o_ca")
    for h in range(NH):
        hs = slice(h * DH, (h + 1) * DH)
        qt = sb.tile([DH, NT], BF16, tag="qk", bufs=2, name=f"qtca_{h}")
        for ncq in range(NC):
            ncs = slice(ncq * NW, (ncq + 1) * NW)
            pq = ps.tile([DH, NW], F32, tag="bank", bufs=8, name=f"pqca_{h}_{ncq}")
            for c in range(KC):
                nc.tensor.matmul(pq, w_ca_q[:, c, hs], x1T[:, c, ncs],
                                 start=(c == 0), stop=(c == KC - 1))
            nc.any.tensor_copy(qt[:, ncs], pq)
        attn_inner(
            h, qt,
            kt=lambda mc, _h=h: kt_ca[:, _h, :],
            vaug_sl=lambda mc, _h=h: vca_aug[:, _h, :],
            o_tile=o_ca, mchunks=1, mpart=CM, zslot=NH * NC + h * NC)

    # ---------------- CA out-proj + LN2, interleaved per token chunk ---------
    r2T = resid_tile("r2T")
    for ncq in range(NC):
        out_proj(ncq, w_ca_o, o_ca, b_ca_bo, x1T, r2T, "r2")
        layernorm_nc(r2T, 2, ncq)
    x2T = r2T

    # ---------------- FF (GEGLU) ----------------
    w_ff2 = []
    for t in range(4):
        wt = sb.tile([P, KC, D], BF16, tag="w640", bufs=4, name=f"w_ff2_{t}")
        nc.gpsimd.dma_start(
            wt, d["ff_w2"].rearrange("(t c p) m -> t p c m", p=P, c=KC)[t])
        w_ff2.append(wt)

    r3T = resid_tile("r3T")
    for ncq in range(NC):
        ncs = slice(ncq * NW, (ncq + 1) * NW)
        mfull = sb.tile([P, FJ, NW], BF16, tag="mfull", bufs=1, name=f"mfull_{ncq}")
        for j in range(FJ):
            wja = sb.tile([P, KC, P], F32R, tag="wff1", bufs=4, name=f"wja_{ncq}_{j}")
            nc.gpsimd.dma_start(
                wja, d["ff_w1"].rearrange("(c p) m -> p c m", p=P)
                [:, :, j * P:(j + 1) * P])
            wjg = sb.tile([P, KC, P], F32R, tag="wff1", bufs=4, name=f"wjg_{ncq}_{j}")
            nc.gpsimd.dma_start(
                wjg, d["ff_w1"].rearrange("(c p) m -> p c m", p=P)
                [:, :, FH + j * P:FH + (j + 1) * P])
            pa = bank(f"pa_{ncq}_{j}")
            pg = bank(f"pg_{ncq}_{j}")
            for c in range(KC):
                nc.tensor.matmul(pa, wja[:, c, :], x2T[:, c, ncs],
                                 start=(c == 0), stop=(c == KC - 1))
            for c in range(KC):
                nc.tensor.matmul(pg, wjg[:, c, :], x2T[:, c, ncs],
                                 start=(c == 0), stop=(c == KC - 1))
            gj = sb.tile([P, NW], BF16, tag="gelu", bufs=2, name=f"gj_{ncq}_{j}")
            nc.scalar.activation(gj, pg, AF.Gelu, bias=b_f1g[:, j:j + 1])
            nc.vector.scalar_tensor_tensor(
                out=mfull[:, j, :], in0=pa, scalar=b_f1a[:, j:j + 1],
                in1=gj, op0=OP.add, op1=OP.mult)
        for do in range(KC):
            dos = slice(do * P, (do + 1) * P)
            pr = bank(f"pr3_{do}_{ncq}")
            for j in range(FJ):
                nc.tensor.matmul(pr, w_ff2[j // KC][:, j % KC, dos],
                                 mfull[:, j, :],
                                 start=(j == 0), stop=(j == FJ - 1))
            nc.vector.scalar_tensor_tensor(
                out=r3T[:, do, ncs], in0=pr, scalar=b_ff2[:, do:do + 1],
                in1=x2T[:, do, ncs].bitcast(F32), op0=OP.add, op1=OP.add)
        layernorm_nc(r3T, 3, ncq)
        for c in range(KC):
            nc.sync.dma_start(
                d["outT"].rearrange("(c p) n -> p c n", p=P)[:, c, ncs],
                r3T[:, c, ncs].bitcast(F32))


def _build(apply_gb):
    nc = bacc.Bacc(None, target_bir_lowering=False)
    dt_in = [
        ("xT", [D, NT], F32R), ("ctxT_bf", [CD, CM], BF16),
        ("sa_wq", [D, D], F32R), ("sa_wk", [D, D], F32R),
        ("sa_wv", [D, D], F32R), ("sa_wo_h", [DH, NH, D], BF16),
        ("ca_wq", [D, D], F32R), ("ca_wk", [CD, D], BF16),
        ("ca_wv", [CD, D], BF16), ("ca_wo_h", [DH, NH, D], BF16),
        ("ff_w1", [D, 2 * FH], F32R), ("ff_w2", [FH, D], BF16),
        ("sa_bo_p", [P, KC], F32), ("ca_bo_p", [P, KC], F32),
        ("ff_b2_p", [P, KC], F32),
        ("ff_b1a_p", [P, FJ], F32), ("ff_b1g_p", [P, FJ], F32),
        ("ones", [P, 1], F32R), ("onesrow", [1, P], F32R), ("epst", [P, 1], F32),
    ]
    if apply_gb:
        for ln in (1, 2, 3):
            dt_in.append((f"ln{ln}_g_p", [P, KC], F32))
            dt_in.append((f"ln{ln}_b_p", [P, KC], F32))
    nc._kd = {}
    for name, shape, dt in dt_in:
        nc._kd[name] = nc.declare_dram_parameter(name, shape, dt,
                                                 isOutput=False)
    nc._kd["outT"] = nc.declare_dram_parameter("outT", [D, NT], F32,
                                               isOutput=True)
    with tile.TileContext(nc) as tc:
        _emit(nc, tc, apply_gb)
    nc.compile()
    return nc


def _prep_in_maps(inputs, apply_gb):
    f32 = np.float32
    bf = ml_dtypes.bfloat16
    x = np.asarray(inputs["x"], f32)
    ctx = np.asarray(inputs["context"], f32)

    def heads(w):
        # [640, 640] -> [80, 8, 640] head-major partition layout
        return np.ascontiguousarray(
            np.asarray(w, f32).reshape(NH, DH, D).transpose(1, 0, 2)
        ).astype(bf)

    def part(v, cols):
        return np.ascontiguousarray(np.asarray(v, f32).reshape(cols, P).T)

    shared = {
        "sa_wq": np.asarray(inputs["sa_wq"], f32),
        "sa_wk": np.asarray(inputs["sa_wk"], f32),
        "sa_wv": np.asarray(inputs["sa_wv"], f32),
        "sa_wo_h": heads(inputs["sa_wo"]),
        "ca_wq": np.asarray(inputs["ca_wq"], f32),
        "ca_wk": np.asarray(inputs["ca_wk"], f32).astype(bf),
        "ca_wv": np.asarray(inputs["ca_wv"], f32).astype(bf),
        "ca_wo_h": heads(inputs["ca_wo"]),
        "ff_w1": np.asarray(inputs["ff_w1"], f32),
        "ff_w2": np.asarray(inputs["ff_w2"], f32).astype(bf),
        "sa_bo_p": part(inputs["sa_bo"], KC),
        "ca_bo_p": part(inputs["ca_bo"], KC),
        "ff_b2_p": part(inputs["ff_b2"], KC),
        "ff_b1a_p": part(np.asarray(inputs["ff_b1"], f32)[:FH], FJ),
        "ff_b1g_p": part(np.asarray(inputs["ff_b1"], f32)[FH:], FJ),
        "ones": np.ones((P, 1), f32),
        "onesrow": np.ones((1, P), f32),
        "epst": np.full((P, 1), LN_EPS, f32),
    }
    if apply_gb:
        for ln in (1, 2, 3):
            shared[f"ln{ln}_g_p"] = part(inputs[f"ln{ln}_g"], KC)
            shared[f"ln{ln}_b_p"] = part(inputs[f"ln{ln}_b"], KC)
    maps = []
    for i in range(B):
        m = dict(shared)
        m["xT"] = np.ascontiguousarray(x[i].T)
        m["ctxT_bf"] = np.ascontiguousarray(ctx[i].T).astype(bf)
        maps.append(m)
    return maps


def _needs_gb(inputs):
    for ln in (1, 2, 3):
        if not np.allclose(np.asarray(inputs[f"ln{ln}_g"]), 1.0):
            return True
        if not np.allclose(np.asarray(inputs[f"ln{ln}_b"]), 0.0):
            return True
    return False


def _run(inputs, trace=False):
    apply_gb = _needs_gb(inputs)
    nc = _build(apply_gb)
    maps = _prep_in_maps(inputs, apply_gb)
    res = run_bass_kernel_spmd(nc, maps, core_ids=list(range(B)), trace=trace)
    out = np.stack([np.asarray(r["outT"]).T for r in res.results])
    return out.astype(np.float32), res


def kernel(**inputs):
    out, _ = _run(inputs, trace=False)
    return out

